# revision 14
# baseline (speedup 1.0000x reference)
"""Multi-head GAT layer (2 heads, sum-merged) on 8 TRN2 NeuronCores.

Strategy: edges are sharded by destination node (12500 dsts per core), so
the segment softmax and scatter-sum are entirely core-local (no
collectives). Node features and weights are replicated; every core
computes the full projected-source table Z = [z | s_src] once, then
processes only its own edges via indirect-DMA gathers. All data-dependent
structure (edge->slot assignment, output rows) is carried in index
tensors, so the compiled program is identical across cores (SPMD).
"""

import numpy as np
import ml_dtypes

import concourse.bass as bass
import concourse.bacc as bacc
import concourse.mybir as mybir
import concourse.tile as tile
from concourse.bass_utils import run_bass_kernel_spmd

F32 = mybir.dt.float32
BF16 = mybir.dt.bfloat16
I32 = mybir.dt.int32

IN = 128          # input feature dim
OUT = 64          # output feature dim per head
H = 2             # heads
ZC = IN + 4       # z-row: 128 z cols + 2 s_src cols + 2 pad = 132
NCORES = 8
K = 8             # edge chunks (of 128) per supertile
CAP = 128 * K     # edge capacity per supertile

N_SRC = 100000
N_DST = 100000
NDST_C = N_DST // NCORES            # 12500 dsts per core
SRC_TILES = 784                     # 784*128 = 100352 >= N_SRC
SRC_PAD = SRC_TILES * 128
SRC_GROUP = 16                      # src tiles per load group (49 groups)
DST_TILES = 98                      # 98*128 = 12544 >= NDST_C
DST_PAD = DST_TILES * 128
DST_GROUP = 7                       # dst tiles per load group (14 groups)
OUT_ROWS = DST_PAD + 128            # trailing 128 rows catch garbage


def _pack_core(src_c, dst_local, dst_pad, cap=None, k=None):
    cap = CAP if cap is None else cap
    k = K if k is None else k
    """Pack one core's edges (dst-sorted) into supertiles.

    Returns list of per-tile dicts with slot-index arrays. Each supertile
    holds whole dst segments only, <= cap edges, dst span <= 128.
    """
    order = np.argsort(dst_local, kind="stable")
    s = np.ascontiguousarray(src_c[order])
    d = np.ascontiguousarray(dst_local[order])
    n = len(d)
    tiles = []
    if n:
        starts = np.flatnonzero(np.r_[True, np.diff(d) != 0])
        ends = np.r_[starts[1:], n]
        segd = d[starts]
        nseg = len(starts)
        cur = 0
        while cur < nseg:
            d0 = int(segd[cur])
            elo = int(starts[cur])
            assert int(ends[cur]) - elo <= cap, "segment larger than supertile"
            hi = cur
            while (
                hi + 1 < nseg
                and int(ends[hi + 1]) - elo <= cap
                and int(segd[hi + 1]) - d0 < 128
            ):
                hi += 1
            tiles.append((d0, elo, int(ends[hi])))
            cur = hi + 1

    out = []
    for d0, elo, ehi in tiles:
        cnt = ehi - elo
        e = np.arange(cnt)
        p, j = e // k, e % k
        gidx = np.zeros((128, k), np.int32)
        dstrel = np.full((128, k), -1, np.int32)
        sdidx = np.zeros((128, k), np.int32)
        gidx[p, j] = s[elo:ehi]
        dstrel[p, j] = d[elo:ehi] - d0
        sdidx[p, j] = d[elo:ehi]
        span = int(d[ehi - 1]) - d0 + 1
        rows = d0 + np.arange(128, dtype=np.int32)
        rows[span:] = dst_pad + np.arange(span, 128, dtype=np.int32)
        out.append((gidx, dstrel, sdidx, rows))
    return out


def _pack_all(src_idx, dst_idx):
    """Pack every core's edges; pad to a common supertile count T."""
    ncores, ndst_c, dst_pad = NCORES, NDST_C, DST_PAD
    per_core = []
    core_of = dst_idx // ndst_c
    for c in range(ncores):
        m = core_of == c
        per_core.append(_pack_core(src_idx[m], dst_idx[m] - c * ndst_c, dst_pad))
    T = max(len(t) for t in per_core)
    eidx = np.zeros((ncores, T, 128, 3 * K + 1), np.int32)
    dummy_rows = dst_pad + np.arange(128, dtype=np.int32)
    for c in range(ncores):
        for ti in range(T):
            if ti < len(per_core[c]):
                gidx, dstrel, sdidx, rows = per_core[c][ti]
            else:
                gidx = np.zeros((128, K), np.int32)
                dstrel = np.full((128, K), -1, np.int32)
                sdidx = np.zeros((128, K), np.int32)
                rows = dummy_rows
            eidx[c, ti, :, 0:K] = gidx
            eidx[c, ti, :, K:2 * K] = dstrel
            eidx[c, ti, :, 2 * K:3 * K] = sdidx
            eidx[c, ti, :, 3 * K] = rows
    return eidx, T


def _build_program(T):
    nc = bacc.Bacc("TRN2", target_bir_lowering=False, debug=False,
                   num_devices=NCORES)
    hsT = nc.dram_tensor("hsrcT", [128, SRC_PAD], BF16, kind="ExternalInput").ap()
    hdT = nc.dram_tensor("hdstT", [128, DST_PAD], BF16, kind="ExternalInput").ap()
    wsr = nc.dram_tensor("wsrc", [128, ZC], BF16, kind="ExternalInput").ap()
    wds = nc.dram_tensor("wdst", [128, 2], BF16, kind="ExternalInput").ap()
    eix = nc.dram_tensor("eidx", [T, 128, 3 * K + 1], I32, kind="ExternalInput").ap()
    Z = nc.dram_tensor("Z", [SRC_PAD, ZC], F32, kind="Internal").ap()
    SD = nc.dram_tensor("SD", [OUT_ROWS, 2], F32, kind="Internal").ap()
    out = nc.dram_tensor("out", [OUT_ROWS, OUT], F32, kind="ExternalOutput").ap()

    from concourse.masks import make_identity

    with tile.TileContext(nc) as tc:
        with (
            tc.tile_pool(name="const", bufs=1) as cpool,
            tc.tile_pool(name="pa", bufs=3) as pa_pool,
            tc.tile_pool(name="pz", bufs=3) as pz_pool,
            tc.tile_pool(name="sda", bufs=1) as sd_pool,
            tc.tile_pool(name="ei", bufs=8) as ei_pool,
            tc.tile_pool(name="zg", bufs=6) as zg_pool,
            tc.tile_pool(name="oht", bufs=6) as oht_pool,
            tc.tile_pool(name="wt", bufs=6) as w_pool,
            tc.tile_pool(name="fl", bufs=6) as f_pool,
        ):
            wsrc_t = cpool.tile([128, ZC], BF16)
            nc.sync.dma_start(out=wsrc_t[:], in_=wsr[:, :])
            wdst_t = cpool.tile([128, 2], BF16)
            nc.sync.dma_start(out=wdst_t[:], in_=wds[:, :])
            iota_t = cpool.tile([128, K * 128], I32)
            nc.gpsimd.iota(iota_t[:], [[0, K], [1, 128]], channel_multiplier=0)
            iop_t = cpool.tile([128, 1], BF16)
            nc.gpsimd.iota(iop_t[:], [[0, 1]], channel_multiplier=1,
                           allow_small_or_imprecise_dtypes=True)
            ident_t = cpool.tile([128, 128], BF16)
            make_identity(nc, ident_t[:])
            zpad_t = cpool.tile([128, 2], F32)
            nc.gpsimd.memset(zpad_t[:], 0.0)
            nc.sync.dma_start(out=SD[DST_PAD:OUT_ROWS, :], in_=zpad_t[:])

            # ---- Phase A: Z = [z | s_src] for all src nodes ----
            with (
                tc.tile_pool(name="psA", bufs=4, space="PSUM") as psA_pool,
                tc.tile_pool(name="psD", bufs=2, space="PSUM") as psD_pool,
            ):
                for g in range(SRC_TILES // SRC_GROUP):
                    hT = pa_pool.tile([128, SRC_GROUP * 128], BF16)
                    nc.scalar.dma_start(
                        out=hT[:],
                        in_=hsT[:, g * SRC_GROUP * 128:(g + 1) * SRC_GROUP * 128])
                    zbig = pz_pool.tile([128, SRC_GROUP * ZC], F32)
                    for j in range(SRC_GROUP):
                        ps = psA_pool.tile([128, ZC], F32)
                        nc.tensor.matmul(
                            out=ps[:], lhsT=hT[:, j * 128:(j + 1) * 128],
                            rhs=wsrc_t[:], start=True, stop=True)
                        nc.vector.tensor_copy(
                            out=zbig[:, j * ZC:(j + 1) * ZC], in_=ps[:])
                    rows = slice(g * SRC_GROUP * 128, (g + 1) * SRC_GROUP * 128)
                    nc.sync.dma_start(
                        out=Z[rows, :].rearrange("(j p) c -> p j c", p=128),
                        in_=zbig[:].rearrange("p (j c) -> p j c", c=ZC))

                # ---- Phase A: s_dst for this core's dst shard ----
                sdall = sd_pool.tile([128, 2 * DST_TILES], F32)
                for g in range(DST_TILES // DST_GROUP):
                    hTd = pa_pool.tile([128, DST_GROUP * 128], BF16, tag="hTd")
                    nc.scalar.dma_start(
                        out=hTd[:],
                        in_=hdT[:, g * DST_GROUP * 128:(g + 1) * DST_GROUP * 128])
                    for j in range(DST_GROUP):
                        t = g * DST_GROUP + j
                        psd = psD_pool.tile([128, 2], F32, tag="psd")
                        nc.tensor.matmul(
                            out=psd[:], lhsT=hTd[:, j * 128:(j + 1) * 128],
                            rhs=wdst_t[:], start=True, stop=True)
                        nc.vector.tensor_copy(
                            out=sdall[:, t * 2:(t + 1) * 2], in_=psd[:])
                nc.sync.dma_start(
                    out=SD[0:DST_PAD, :].rearrange("(t p) c -> p t c", p=128),
                    in_=sdall[:].rearrange("p (t c) -> p t c", c=2))

            # ---- Phase B: edge supertiles ----
            with (
                tc.tile_pool(name="psB", bufs=3, space="PSUM") as psB_pool,
                tc.tile_pool(name="psOH", bufs=3, space="PSUM") as psOH_pool,
                tc.tile_pool(name="psSD", bufs=2, space="PSUM") as psSD_pool,
            ):
              for t in range(T):
                  ei = ei_pool.tile([128, 3 * K + 1], I32)
                  nc.scalar.dma_start(out=ei[:], in_=eix[t, :, :])
                  zg = zg_pool.tile([128, K * ZC], F32)
                  for j in range(K):
                      nc.gpsimd.indirect_dma_start(
                          out=zg[:, j * ZC:(j + 1) * ZC], out_offset=None,
                          in_=Z[:, :],
                          in_offset=bass.IndirectOffsetOnAxis(
                              ap=ei[:, j:j + 1], axis=0))
                  # s_dst for the tile's 128 dst slots (one gather), then
                  # per-edge expansion via one-hot matmuls on the PE.
                  sdslotF = w_pool.tile([128, 2], F32, tag="sdslotF")
                  nc.gpsimd.indirect_dma_start(
                      out=sdslotF[:], out_offset=None, in_=SD[:, :],
                      in_offset=bass.IndirectOffsetOnAxis(
                          ap=ei[:, 3 * K:3 * K + 1], axis=0))
                  sdslot = w_pool.tile([128, 2], BF16, tag="sdslot")
                  nc.vector.tensor_copy(out=sdslot[:], in_=sdslotF[:])
                  drelF = w_pool.tile([128, K], BF16, tag="drelF")
                  nc.vector.tensor_copy(out=drelF[:], in_=ei[:, K:2 * K])
                  sdg = w_pool.tile([128, K * 2], F32, tag="sdg")
                  for j in range(K):
                      ps_oh = psOH_pool.tile([128, 128], BF16, tag="psoh")
                      nc.tensor.transpose(
                          out=ps_oh[:],
                          in_=drelF[:, j:j + 1].to_broadcast([128, 128]),
                          identity=ident_t[:])
                      oh_s = oht_pool.tile([128, 128], BF16, tag="ohs")
                      nc.vector.tensor_tensor(
                          out=oh_s[:], in0=iop_t[:, 0:1].to_broadcast([128, 128]),
                          in1=ps_oh[:], op=mybir.AluOpType.is_equal)
                      ps_sd = psSD_pool.tile([128, 2], F32, tag="pssd")
                      nc.tensor.matmul(out=ps_sd[:], lhsT=oh_s[:],
                                       rhs=sdslot[:], start=True, stop=True)
                      nc.vector.tensor_copy(
                          out=sdg[:, j * 2:(j + 1) * 2], in_=ps_sd[:])
                  oht = oht_pool.tile([128, K * 128], BF16)
                  nc.vector.tensor_tensor(
                      out=oht[:],
                      in0=ei[:, K:2 * K].to_broadcast([128, K, 128]),
                      in1=iota_t[:].rearrange("p (k q) -> p k q", q=128),
                      op=mybir.AluOpType.is_equal)
                  zg3 = zg[:].rearrange("p (j c) -> p j c", c=ZC)
                  st = w_pool.tile([128, K * 2], F32, tag="st")
                  nc.vector.tensor_tensor(
                      out=st[:].rearrange("p (j c) -> p j c", c=2),
                      in0=zg3[:, :, IN:IN + 2],
                      in1=sdg[:].rearrange("p (j c) -> p j c", c=2),
                      op=mybir.AluOpType.add)
                  st2 = w_pool.tile([128, K * 2], F32, tag="st2")
                  nc.vector.tensor_scalar_mul(out=st2[:], in0=st[:], scalar1=0.01)
                  nc.vector.tensor_tensor(
                      out=st[:], in0=st[:], in1=st2[:], op=mybir.AluOpType.max)
                  wt = w_pool.tile([128, K * 2], F32, tag="wt")
                  nc.scalar.activation(
                      out=wt[:], in_=st[:], func=mybir.ActivationFunctionType.Exp)
                  wt3 = wt[:].rearrange("p (j c) -> p j c", c=2)
                  wzb = zg_pool.tile([128, K * ZC], BF16, tag="wzb")
                  wzb3 = wzb[:].rearrange("p (j c) -> p j c", c=ZC)
                  for h in range(H):
                      nc.vector.tensor_tensor(
                          out=wzb3[:, :, h * OUT:(h + 1) * OUT],
                          in0=zg3[:, :, h * OUT:(h + 1) * OUT],
                          in1=wt3[:, :, h:h + 1].to_broadcast([128, K, OUT]),
                          op=mybir.AluOpType.mult)
                  nc.vector.tensor_copy(out=wzb3[:, :, IN:IN + 2], in_=wt3[:, :, :])
                  ps = psB_pool.tile([128, ZC], F32)
                  for j in range(K):
                      nc.tensor.matmul(
                          out=ps[:, 0:IN + 2],
                          lhsT=oht[:, j * 128:(j + 1) * 128],
                          rhs=wzb[:, j * ZC:j * ZC + IN + 2],
                          start=(j == 0), stop=(j == K - 1))
                  den = f_pool.tile([128, 2], F32, tag="den")
                  nc.vector.tensor_scalar_max(
                      out=den[:], in0=ps[:, IN:IN + 2], scalar1=1e-30)
                  rec = f_pool.tile([128, 2], F32, tag="rec")
                  nc.vector.reciprocal(out=rec[:], in_=den[:])
                  o0 = f_pool.tile([128, OUT], F32, tag="o0")
                  nc.vector.tensor_scalar_mul(
                      out=o0[:], in0=ps[:, 0:OUT], scalar1=rec[:, 0:1])
                  ot = f_pool.tile([128, OUT], F32, tag="ot")
                  nc.vector.tensor_scalar_mul(
                      out=ot[:], in0=ps[:, OUT:2 * OUT], scalar1=rec[:, 1:2])
                  nc.vector.tensor_add(out=ot[:], in0=ot[:], in1=o0[:])
                  nc.gpsimd.indirect_dma_start(
                      out=out[:, :],
                      out_offset=bass.IndirectOffsetOnAxis(
                          ap=ei[:, 3 * K:3 * K + 1], axis=0),
                      in_=ot[:], in_offset=None)

    nc.compile()
    return nc


def _prep_inputs(h_src, h_dst, W_src, W_dst, a_w, src_idx, dst_idx):
    """Host-side sharding/layout prep. Returns in_maps for the 8 cores."""
    hs = np.zeros((SRC_PAD, IN), np.float32)
    hs[:N_SRC] = h_src
    hsrcT = np.ascontiguousarray(hs.T.astype(ml_dtypes.bfloat16))

    # wsrc: [IN, ZC] = [ W[h,o,d] at col h*OUT+o | w~_s | pad ]
    wsr = np.zeros((IN, ZC), np.float32)
    wsr[:, :H * OUT] = W_src.reshape(H * OUT, IN).T
    a_s, a_d = a_w[:, :OUT], a_w[:, OUT:]
    wsr[:, H * OUT:H * OUT + H] = np.einsum("hod,ho->dh", W_src, a_s)
    wsr = wsr.astype(ml_dtypes.bfloat16)
    wds = np.einsum("hod,ho->dh", W_dst, a_d).astype(ml_dtypes.bfloat16)

    eidx, T = _pack_all(src_idx, dst_idx)

    in_maps = []
    for c in range(NCORES):
        hd = np.zeros((DST_PAD, IN), np.float32)
        hd[:NDST_C] = h_dst[c * NDST_C:(c + 1) * NDST_C]
        hdstT = np.ascontiguousarray(hd.T.astype(ml_dtypes.bfloat16))
        in_maps.append({
            "hsrcT": hsrcT,
            "hdstT": hdstT,
            "wsrc": wsr,
            "wdst": wds,
            "eidx": eidx[c],
        })
    return in_maps, T


def _run(inputs, trace=False):
    inputs = {k: np.asarray(v) for k, v in inputs.items()}
    in_maps, T = _prep_inputs(**inputs)
    nc = _build_program(T)
    res = run_bass_kernel_spmd(
        nc, in_maps, core_ids=list(range(NCORES)), trace=trace)
    parts = [res.results[c]["out"][:NDST_C] for c in range(NCORES)]
    return np.concatenate(parts, axis=0), res


def kernel(**inputs):
    out, _ = _run(inputs, trace=False)
    return out


# revision 15
# speedup vs baseline: 1.1867x; 1.1867x over previous
"""Multi-head GAT layer (2 heads, sum-merged) on 8 TRN2 NeuronCores.

Edges are sharded by destination node (12500 dsts/core): segment softmax
and scatter-sum stay core-local (no collectives). Every core builds the
projected-source table Z = [z | s_src | pad] (256-col bf16 rows) once.

v5: per-edge rows are fetched with batched Q7 dma_gather (int16 indices,
Z split into 4 zero-offset quarter tensors of 26624 rows; gathers of 512
indices each) instead of per-chunk indirect DMAs -- removing the ~1ms of
per-instruction SWDGE time on the Pool engine that bounds the baseline.
Per-edge s_dst is precomputed in phase A from a host-expanded h_dst[dst_e]
operand (slot order), so phase B needs no s_dst gather or one-hot
expansion. Supertile outputs go to a slot-ordered bf16 `big` buffer via
static direct DMAs; a final set of dma_gathers (256B rows) remaps slots
to dst rows.
"""

import numpy as np
import ml_dtypes

import concourse.bacc as bacc
import concourse.mybir as mybir
import concourse.tile as tile
from concourse.bass_utils import run_bass_kernel_spmd

F32 = mybir.dt.float32
BF16 = mybir.dt.bfloat16
I16 = mybir.dt.int16

IN = 128          # input feature dim
OUT = 64          # output feature dim per head
H = 2             # heads
ZC = 256          # Z row cols: 128 z + 2 s_src + 126 pad (512B rows)
NCORES = 8
K = 8             # chunks (of 128 slots) per supertile
QCAP = 256        # edge slots per supertile per src-quarter (2 chunks)
NQ = 4            # Z quarters
QROWS = 26624     # rows per quarter (13 groups of 2048; 4*26624 = 106496)
B = 8             # supertiles per gather block
GNI = 512         # indices per dma_gather (hw-proven size)

N_SRC = 100000
N_DST = 100000
NDST_C = N_DST // NCORES
SRC_TILES = 832
SRC_PAD = SRC_TILES * 128           # 106496
SRC_GROUP = 16
OUT_ROWS = 12800                    # 25*512 gather indices; >= 12500
BIGC = 128                          # big row cols (64 used; 256B bf16 rows)


def _pack_core(src_c, dst_local):
    """dst-sorted edges -> supertiles of whole dst segments with
    total<=1024, dst span<128, and <=QCAP edges per src quarter."""
    order = np.argsort(dst_local, kind="stable")
    s = np.ascontiguousarray(src_c[order])
    d = np.ascontiguousarray(dst_local[order])
    n = len(d)
    starts = np.flatnonzero(np.r_[True, np.diff(d) != 0])
    ends = np.r_[starts[1:], n]
    segd = d[starts]
    nseg = len(starts)
    tiles = []
    cur = 0
    while cur < nseg:
        d0 = int(segd[cur])
        elo = int(starts[cur])
        qcnt = np.zeros(NQ, np.int64)
        hi = cur - 1
        while hi + 1 < nseg and int(segd[hi + 1]) - d0 < 128:
            nlo, nhi = int(starts[hi + 1]), int(ends[hi + 1])
            if nhi - elo > K * 128:
                break
            qs = np.bincount(s[nlo:nhi] // QROWS, minlength=NQ)
            if np.any(qcnt + qs > QCAP):
                break
            qcnt += qs
            hi += 1
        assert hi >= cur, "single segment violates caps"
        tiles.append((d0, elo, int(ends[hi])))
        cur = hi + 1
    out = []
    for d0, elo, ehi in tiles:
        ss, dd = s[elo:ehi], d[elo:ehi]
        q = ss // QROWS
        pos = np.empty(len(ss), np.int64)
        qidx = np.zeros((NQ, QCAP), np.int16)   # relative row ids (pad 0)
        for qq in range(NQ):
            m = np.flatnonzero(q == qq)
            pos[m] = qq * QCAP + np.arange(len(m))
            qidx[qq, :len(m)] = (ss[m] - qq * QROWS).astype(np.int16)
        jj, pp = pos // 128, pos % 128          # chunk, partition
        dstrel = np.full((128, K), -1, np.float32)
        dstrel[pp, jj] = dd - d0
        dcol = np.full((128, K), -1, np.int64)  # local dst per slot
        dcol[pp, jj] = dd
        out.append(dict(d0=d0, qidx=qidx, dstrel=dstrel, dcol=dcol))
    return out


def _wrap16(idx_list):
    """int16 idx list -> [128, n/16] wrapped in 16 partitions, x8 cores."""
    n = len(idx_list)
    iw = np.zeros((128, n // 16), np.int16)
    base = idx_list.reshape(n // 16, 16).T      # [16, n/16]
    for rep in range(8):
        iw[rep * 16:(rep + 1) * 16] = base
    return iw


def _pack_all(src_idx, dst_idx):
    per_core = []
    core_of = dst_idx // NDST_C
    for c in range(NCORES):
        m = core_of == c
        per_core.append(_pack_core(src_idx[m], dst_idx[m] - c * NDST_C))
    T = max(len(t) for t in per_core)
    NB = (T + B - 1) // B
    T = NB * B
    qidx = np.zeros((NCORES, NB, NQ, 128, (B * QCAP) // 16), np.int16)
    edrel = np.full((NCORES, T, 128, K), -1, np.float32)
    dcol = np.full((NCORES, T, 128, K), -1, np.int64)
    remap = np.zeros((NCORES, OUT_ROWS), np.int16)
    for c in range(NCORES):
        tiles = per_core[c]
        for b in range(NB):
            for qq in range(NQ):
                lst = np.zeros(B * QCAP, np.int16)
                for s in range(B):
                    ti = b * B + s
                    if ti < len(tiles):
                        lst[s * QCAP:(s + 1) * QCAP] = tiles[ti]["qidx"][qq]
                qidx[c, b, qq] = _wrap16(lst)
        for ti, t in enumerate(tiles):
            edrel[c, ti] = t["dstrel"]
            dcol[c, ti] = t["dcol"]
            d0 = t["d0"]
            span = min(128, NDST_C - d0)
            rows = d0 + np.arange(span)
            remap[c, rows] = (ti * 128 + np.arange(span)).astype(np.int16)
    return qidx, edrel.astype(ml_dtypes.bfloat16), dcol, remap, T, NB


def _build_program(T, NB):
    nc = bacc.Bacc("TRN2", target_bir_lowering=False, debug=False,
                   num_devices=NCORES)
    hsT = nc.dram_tensor("hsrcT", [128, SRC_PAD], BF16, kind="ExternalInput").ap()
    hdE = nc.dram_tensor("hdE", [128, T * K * 128], BF16, kind="ExternalInput").ap()
    wsr = nc.dram_tensor("wsrc", [128, 132], BF16, kind="ExternalInput").ap()
    wds = nc.dram_tensor("wdst", [128, 2], BF16, kind="ExternalInput").ap()
    qix = nc.dram_tensor("qidx", [NB, NQ, 128, (B * QCAP) // 16], I16,
                         kind="ExternalInput").ap()
    edr = nc.dram_tensor("edrel", [T, 128, K], BF16, kind="ExternalInput").ap()
    rmp = nc.dram_tensor("remap", [128, OUT_ROWS // 16], I16,
                         kind="ExternalInput").ap()
    Zq = [nc.dram_tensor(f"Z{q}", [QROWS, ZC], BF16, kind="Internal").ap()
          for q in range(NQ)]
    big = nc.dram_tensor("big", [T * 128, BIGC], BF16, kind="Internal").ap()
    out = nc.dram_tensor("out", [OUT_ROWS, OUT], BF16, kind="ExternalOutput").ap()

    AF = mybir.ActivationFunctionType
    ALU = mybir.AluOpType
    NSLOT = B * QCAP                 # 2048 slots per quarter per block

    with tile.TileContext(nc) as tc:
        with (
            tc.tile_pool(name="const", bufs=1) as cpool,
            tc.tile_pool(name="pa", bufs=3) as pa_pool,
            tc.tile_pool(name="pad", bufs=2) as pad_pool,
            tc.tile_pool(name="pz", bufs=3) as pz_pool,
            tc.tile_pool(name="sde", bufs=1) as sde_pool,
            tc.tile_pool(name="qi", bufs=3) as qi_pool,
            tc.tile_pool(name="ed", bufs=3) as ed_pool,
            tc.tile_pool(name="zg", bufs=2) as zg_pool,
            tc.tile_pool(name="wz", bufs=3) as wz_pool,
            tc.tile_pool(name="oh", bufs=3) as oh_pool,
            tc.tile_pool(name="wt", bufs=4) as w_pool,
            tc.tile_pool(name="fl", bufs=4) as f_pool,
            tc.tile_pool(name="ob", bufs=4) as ob_pool,
            tc.tile_pool(name="fg", bufs=1) as fg_pool,
        ):
            wsrc_t = cpool.tile([128, 132], BF16)
            nc.sync.dma_start(out=wsrc_t[:], in_=wsr[:, :])
            wdst_t = cpool.tile([128, 2], BF16)
            nc.sync.dma_start(out=wdst_t[:], in_=wds[:, :])
            iota_t = cpool.tile([128, K * 128], BF16)
            nc.gpsimd.iota(iota_t[:], [[0, K], [1, 128]], channel_multiplier=0,
                           allow_small_or_imprecise_dtypes=True)

            # ---- Phase A1: per-edge s_dst (host pre-expanded h columns) ----
            NC8 = T * K
            sde = sde_pool.tile([128, NC8 * 2], BF16)
            with tc.tile_pool(name="psD", bufs=4, space="PSUM") as psD_pool:
                ng = (NC8 + 63) // 64
                for g in range(ng):
                    lo, hi = g * 64, min((g + 1) * 64, NC8)
                    hTd = pad_pool.tile([128, 64 * 128], BF16)
                    nc.scalar.dma_start(
                        out=hTd[:, 0:(hi - lo) * 128],
                        in_=hdE[:, lo * 128:hi * 128])
                    psd = psD_pool.tile([128, 128], F32)
                    for j in range(hi - lo):
                        nc.tensor.matmul(
                            out=psd[:, j * 2:(j + 1) * 2],
                            lhsT=hTd[:, j * 128:(j + 1) * 128],
                            rhs=wdst_t[:], start=True, stop=True)
                    nc.vector.tensor_copy(
                        out=sde[:, lo * 2:hi * 2], in_=psd[:, 0:(hi - lo) * 2])

            # ---- Phase A2: Z = [z | s_src | pad] for all src nodes ----
            with tc.tile_pool(name="psA", bufs=4, space="PSUM") as psA_pool:
                ci = 0
                for g in range(SRC_TILES // SRC_GROUP):
                    hT = pa_pool.tile([128, SRC_GROUP * 128], BF16, tag="hT")
                    nc.scalar.dma_start(
                        out=hT[:],
                        in_=hsT[:, g * SRC_GROUP * 128:(g + 1) * SRC_GROUP * 128])
                    zbig = pz_pool.tile([128, SRC_GROUP * ZC], BF16)
                    for j in range(SRC_GROUP):
                        ps = psA_pool.tile([128, 132], F32)
                        nc.tensor.matmul(
                            out=ps[:], lhsT=hT[:, j * 128:(j + 1) * 128],
                            rhs=wsrc_t[:], start=True, stop=True)
                        if ci % 2 == 0:
                            nc.vector.tensor_copy(
                                out=zbig[:, j * ZC:j * ZC + 132], in_=ps[:])
                        else:
                            nc.scalar.copy(
                                out=zbig[:, j * ZC:j * ZC + 132], in_=ps[:])
                        ci += 1
                    qq, gl = g // 13, g % 13
                    rows = slice(gl * SRC_GROUP * 128, (gl + 1) * SRC_GROUP * 128)
                    nc.sync.dma_start(
                        out=Zq[qq][rows, :].rearrange("(j p) c -> p j c", p=128),
                        in_=zbig[:].rearrange("p (j c) -> p j c", c=ZC))

            # ---- Phase B: blocks of 8 supertiles ----
            with tc.tile_pool(name="psB", bufs=6, space="PSUM") as psB_pool:
                for b in range(NB):
                    qit = qi_pool.tile([128, NQ * (NSLOT // 16)], I16)
                    nc.sync.dma_start(
                        out=qit[:].rearrange("p (q m) -> p q m", q=NQ),
                        in_=qix[b, :, :, :].rearrange("q p m -> p q m"))
                    edt = ed_pool.tile([128, B * K], BF16)
                    nc.scalar.dma_start(
                        out=edt[:].rearrange("p (s k) -> p s k", k=K),
                        in_=edr[b * B:(b + 1) * B, :, :].rearrange(
                            "s p k -> p s k"))
                    zg = zg_pool.tile([128, (NSLOT // 128) * NQ * ZC], BF16)
                    zg3 = zg[:].rearrange("p (m c) -> p m c", c=ZC)
                    for qq in range(NQ):
                        for g2 in range(NSLOT // GNI):
                            nc.gpsimd.dma_gather(
                                out_ap=zg3[:, qq * 16 + g2 * 4:
                                           qq * 16 + (g2 + 1) * 4, :],
                                in_ap=Zq[qq][:, :],
                                idxs_ap=qit[:, qq * (NSLOT // 16) + g2 * 32:
                                            qq * (NSLOT // 16) + (g2 + 1) * 32],
                                num_idxs=GNI,
                                num_idxs_reg=GNI,
                                elem_size=ZC,
                            )
                    zg4 = zg[:].rearrange("p (q m c) -> p q m c", q=NQ, c=ZC)
                    for s in range(B):
                        t = b * B + s
                        # chunk j=2q+sub of supertile s -> zg col 16q+s*2+sub
                        sl4 = zg4[:, :, s * 2:s * 2 + 2, :]
                        st = w_pool.tile([128, 2 * K], F32, tag="st")
                        nc.vector.tensor_tensor(
                            out=st[:].rearrange("p (q m c) -> p q m c",
                                                q=NQ, c=2),
                            in0=sl4[:, :, :, IN:IN + 2],
                            in1=sde[:, t * K * 2:(t + 1) * K * 2].rearrange(
                                "p (q m c) -> p q m c", q=NQ, c=2),
                            op=ALU.add)
                        stl = w_pool.tile([128, 2 * K], F32, tag="stl")
                        nc.vector.scalar_tensor_tensor(
                            out=stl[:], in0=st[:], scalar=0.01, in1=st[:],
                            op0=ALU.mult, op1=ALU.max)
                        wt = w_pool.tile([128, 2 * K], BF16, tag="wt")
                        nc.scalar.activation(out=wt[:], in_=stl[:], func=AF.Exp)
                        wt4 = wt[:].rearrange("p (q m c) -> p q m c", q=NQ, c=2)
                        wzb = wz_pool.tile([128, K * 130], BF16)
                        wzb4 = wzb[:].rearrange("p (q m c) -> p q m c",
                                                q=NQ, c=130)
                        for h in range(H):
                            nc.vector.tensor_tensor(
                                out=wzb4[:, :, :, h * OUT:(h + 1) * OUT],
                                in0=sl4[:, :, :, h * OUT:(h + 1) * OUT],
                                in1=wt4[:, :, :, h:h + 1].to_broadcast(
                                    [128, NQ, 2, OUT]),
                                op=ALU.mult)
                        nc.scalar.copy(out=wzb4[:, :, :, IN:IN + 2], in_=wt4[:])
                        oht = oh_pool.tile([128, K * 128], BF16)
                        nc.vector.tensor_tensor(
                            out=oht[:],
                            in0=edt[:, s * K:(s + 1) * K].to_broadcast(
                                [128, K, 128]),
                            in1=iota_t[:].rearrange("p (k q) -> p k q", q=128),
                            op=ALU.is_equal)
                        ps = psB_pool.tile([128, 130], F32)
                        for j in range(K):
                            nc.tensor.matmul(
                                out=ps[:],
                                lhsT=oht[:, j * 128:(j + 1) * 128],
                                rhs=wzb[:, j * 130:(j + 1) * 130],
                                start=(j == 0), stop=(j == K - 1))
                        den = f_pool.tile([128, 2], F32, tag="den")
                        nc.vector.tensor_scalar_max(
                            out=den[:], in0=ps[:, IN:IN + 2], scalar1=1e-30)
                        rec = f_pool.tile([128, 2], F32, tag="rec")
                        nc.vector.reciprocal_approx_fast(out=rec[:], in_=den[:])
                        o0 = f_pool.tile([128, OUT], F32, tag="o0")
                        nc.scalar.mul(o0[:], ps[:, 0:OUT], rec[:, 0:1])
                        ob = ob_pool.tile([128, BIGC], BF16)
                        nc.vector.scalar_tensor_tensor(
                            out=ob[:, 0:OUT], in0=ps[:, OUT:2 * OUT],
                            scalar=rec[:, 1:2], in1=o0[:],
                            op0=ALU.mult, op1=ALU.add)
                        nc.sync.dma_start(
                            out=big[t * 128:(t + 1) * 128, 0:OUT],
                            in_=ob[:, 0:OUT])

                # ---- final: remap slot rows -> dst rows ----
                rmt = qi_pool.tile([128, OUT_ROWS // 16], I16, tag="rmt")
                nc.sync.dma_start(out=rmt[:], in_=rmp[:, :])
                fg = fg_pool.tile([128, (OUT_ROWS // 128) * BIGC], BF16)
                fg3 = fg[:].rearrange("p (m c) -> p m c", c=BIGC)
                for g in range(OUT_ROWS // GNI):
                    nc.gpsimd.dma_gather(
                        out_ap=fg3[:, g * 4:(g + 1) * 4, :],
                        in_ap=big[:, :],
                        idxs_ap=rmt[:, g * 32:(g + 1) * 32],
                        num_idxs=GNI,
                        num_idxs_reg=GNI,
                        elem_size=BIGC,
                    )
                nc.sync.dma_start(
                    out=out[:, :].rearrange("(m p) c -> p m c", p=128),
                    in_=fg3[:, :, 0:OUT])

    nc.compile()
    return nc


def _prep_inputs(h_src, h_dst, W_src, W_dst, a_w, src_idx, dst_idx):
    hs = np.zeros((SRC_PAD, IN), np.float32)
    hs[:N_SRC] = h_src
    hsrcT = np.ascontiguousarray(hs.T.astype(ml_dtypes.bfloat16))

    wsr = np.zeros((IN, 132), np.float32)
    wsr[:, :H * OUT] = W_src.reshape(H * OUT, IN).T
    a_s, a_d = a_w[:, :OUT], a_w[:, OUT:]
    wsr[:, H * OUT:H * OUT + H] = np.einsum("hod,ho->dh", W_src, a_s)
    wsr = wsr.astype(ml_dtypes.bfloat16)
    wds = np.einsum("hod,ho->dh", W_dst, a_d).astype(ml_dtypes.bfloat16)

    qidx, edrel, dcol, remap, T, NB = _pack_all(
        np.asarray(src_idx), np.asarray(dst_idx))

    in_maps = []
    for c in range(NCORES):
        hd = h_dst[c * NDST_C:(c + 1) * NDST_C].astype(np.float32)
        dc = dcol[c].reshape(T, 128, K).transpose(0, 2, 1).reshape(-1)
        hdEc = np.zeros((T * K * 128, IN), np.float32)
        valid = dc >= 0
        hdEc[valid] = hd[dc[valid]]
        hdEc = np.ascontiguousarray(hdEc.T.astype(ml_dtypes.bfloat16))
        rw = _wrap16(remap[c].astype(np.int16))
        in_maps.append({
            "hsrcT": hsrcT,
            "hdE": hdEc,
            "wsrc": wsr,
            "wdst": wds,
            "qidx": qidx[c],
            "edrel": np.ascontiguousarray(edrel[c]),
            "remap": rw,
        })
    return in_maps, T, NB


def _run(inputs, trace=False):
    inputs = {k: np.asarray(v) for k, v in inputs.items()}
    in_maps, T, NB = _prep_inputs(**inputs)
    nc = _build_program(T, NB)
    res = run_bass_kernel_spmd(
        nc, in_maps, core_ids=list(range(NCORES)), trace=trace)
    parts = [np.asarray(res.results[c]["out"][:NDST_C]).astype(np.float32)
             for c in range(NCORES)]
    return np.concatenate(parts, axis=0), res


def kernel(**inputs):
    out, _ = _run(inputs, trace=False)
    return out


# revision 16
# speedup vs baseline: 1.3032x; 1.0982x over previous
"""Multi-head GAT layer (2 heads, sum-merged) on 8 TRN2 NeuronCores.

Edges are sharded by destination node (12500 dsts/core): segment softmax
and scatter-sum stay core-local (no collectives). Every core builds the
projected-source table Z = [z | s_src | pad] (256-col bf16 rows) once.

v5: per-edge rows are fetched with batched Q7 dma_gather (int16 indices,
Z split into 4 zero-offset quarter tensors of 26624 rows; gathers of 512
indices each) instead of per-chunk indirect DMAs -- removing the ~1ms of
per-instruction SWDGE time on the Pool engine that bounds the baseline.
Per-edge s_dst is precomputed in phase A from a host-expanded h_dst[dst_e]
operand (slot order), so phase B needs no s_dst gather or one-hot
expansion. Supertile outputs go to a slot-ordered bf16 `big` buffer via
static direct DMAs; a final set of dma_gathers (256B rows) remaps slots
to dst rows.
"""

import numpy as np
import ml_dtypes

import concourse.bacc as bacc
import concourse.mybir as mybir
import concourse.tile as tile
from concourse.bass_utils import run_bass_kernel_spmd

F32 = mybir.dt.float32
BF16 = mybir.dt.bfloat16
I16 = mybir.dt.int16

IN = 128          # input feature dim
OUT = 64          # output feature dim per head
H = 2             # heads
ZC = 256          # Z row cols: 128 z + 2 s_src + 126 pad (512B rows)
NCORES = 8
K = 8             # chunks (of 128 slots) per supertile
QCAP = 256        # edge slots per supertile per src-quarter (2 chunks)
NQ = 4            # Z quarters
QROWS = 26624     # rows per quarter (13 groups of 2048; 4*26624 = 106496)
B = 8             # supertiles per gather block
GNI = 1024        # indices per dma_gather

N_SRC = 100000
N_DST = 100000
NDST_C = N_DST // NCORES
SRC_TILES = 832
SRC_PAD = SRC_TILES * 128           # 106496
SRC_GROUP = 16
OUT_ROWS = 12800                    # 25*512 gather indices; >= 12500
BIGC = 128                          # big row cols (64 used; 256B bf16 rows)


def _pack_core(src_c, dst_local):
    """dst-sorted edges -> supertiles of whole dst segments with
    total<=1024, dst span<128, and <=QCAP edges per src quarter."""
    order = np.argsort(dst_local, kind="stable")
    s = np.ascontiguousarray(src_c[order])
    d = np.ascontiguousarray(dst_local[order])
    n = len(d)
    starts = np.flatnonzero(np.r_[True, np.diff(d) != 0])
    ends = np.r_[starts[1:], n]
    segd = d[starts]
    nseg = len(starts)
    tiles = []
    cur = 0
    while cur < nseg:
        d0 = int(segd[cur])
        elo = int(starts[cur])
        qcnt = np.zeros(NQ, np.int64)
        hi = cur - 1
        while hi + 1 < nseg and int(segd[hi + 1]) - d0 < 128:
            nlo, nhi = int(starts[hi + 1]), int(ends[hi + 1])
            if nhi - elo > K * 128:
                break
            qs = np.bincount(s[nlo:nhi] // QROWS, minlength=NQ)
            if np.any(qcnt + qs > QCAP):
                break
            qcnt += qs
            hi += 1
        assert hi >= cur, "single segment violates caps"
        tiles.append((d0, elo, int(ends[hi])))
        cur = hi + 1
    out = []
    for d0, elo, ehi in tiles:
        ss, dd = s[elo:ehi], d[elo:ehi]
        q = ss // QROWS
        pos = np.empty(len(ss), np.int64)
        qidx = np.zeros((NQ, QCAP), np.int16)   # relative row ids (pad 0)
        for qq in range(NQ):
            m = np.flatnonzero(q == qq)
            pos[m] = qq * QCAP + np.arange(len(m))
            qidx[qq, :len(m)] = (ss[m] - qq * QROWS).astype(np.int16)
        jj, pp = pos // 128, pos % 128          # chunk, partition
        dstrel = np.full((128, K), -1, np.float32)
        dstrel[pp, jj] = dd - d0
        dcol = np.full((128, K), -1, np.int64)  # local dst per slot
        dcol[pp, jj] = dd
        out.append(dict(d0=d0, qidx=qidx, dstrel=dstrel, dcol=dcol))
    return out


def _wrap16(idx_list):
    """int16 idx list -> [128, n/16] wrapped in 16 partitions, x8 cores."""
    n = len(idx_list)
    iw = np.zeros((128, n // 16), np.int16)
    base = idx_list.reshape(n // 16, 16).T      # [16, n/16]
    for rep in range(8):
        iw[rep * 16:(rep + 1) * 16] = base
    return iw


def _pack_all(src_idx, dst_idx):
    per_core = []
    core_of = dst_idx // NDST_C
    for c in range(NCORES):
        m = core_of == c
        per_core.append(_pack_core(src_idx[m], dst_idx[m] - c * NDST_C))
    T = max(len(t) for t in per_core)
    NB = (T + B - 1) // B
    T = NB * B
    qidx = np.zeros((NCORES, NB, NQ, 128, (B * QCAP) // 16), np.int16)
    edrel = np.full((NCORES, T, 128, K), -1, np.float32)
    dcol = np.full((NCORES, T, 128, K), -1, np.int64)
    remap = np.zeros((NCORES, OUT_ROWS), np.int16)
    for c in range(NCORES):
        tiles = per_core[c]
        for b in range(NB):
            for qq in range(NQ):
                lst = np.zeros(B * QCAP, np.int16)
                for s in range(B):
                    ti = b * B + s
                    if ti < len(tiles):
                        lst[s * QCAP:(s + 1) * QCAP] = tiles[ti]["qidx"][qq]
                qidx[c, b, qq] = _wrap16(lst)
        for ti, t in enumerate(tiles):
            edrel[c, ti] = t["dstrel"]
            dcol[c, ti] = t["dcol"]
            d0 = t["d0"]
            span = min(128, NDST_C - d0)
            rows = d0 + np.arange(span)
            remap[c, rows] = (ti * 128 + np.arange(span)).astype(np.int16)
    return qidx, edrel.astype(ml_dtypes.bfloat16), dcol, remap, T, NB


def _build_program(T, NB):
    nc = bacc.Bacc("TRN2", target_bir_lowering=False, debug=False,
                   num_devices=NCORES)
    hsT = nc.dram_tensor("hsrcT", [128, SRC_PAD], BF16, kind="ExternalInput").ap()
    hdE = nc.dram_tensor("hdE", [128, T * K * 128], BF16, kind="ExternalInput").ap()
    wsr = nc.dram_tensor("wsrc", [128, 132], BF16, kind="ExternalInput").ap()
    wds = nc.dram_tensor("wdst", [128, 2], BF16, kind="ExternalInput").ap()
    qix = nc.dram_tensor("qidx", [NB, NQ, 128, (B * QCAP) // 16], I16,
                         kind="ExternalInput").ap()
    edr = nc.dram_tensor("edrel", [T, 128, K], BF16, kind="ExternalInput").ap()
    rmp = nc.dram_tensor("remap", [128, OUT_ROWS // 16], I16,
                         kind="ExternalInput").ap()
    Zq = [nc.dram_tensor(f"Z{q}", [QROWS, ZC], BF16, kind="Internal").ap()
          for q in range(NQ)]
    big = nc.dram_tensor("big", [T * 128, BIGC], BF16, kind="Internal").ap()
    out = nc.dram_tensor("out", [OUT_ROWS, OUT], BF16, kind="ExternalOutput").ap()

    AF = mybir.ActivationFunctionType
    ALU = mybir.AluOpType
    NSLOT = B * QCAP                 # 2048 slots per quarter per block

    with tile.TileContext(nc) as tc:
        with (
            tc.tile_pool(name="const", bufs=1) as cpool,
            tc.tile_pool(name="pa", bufs=3) as pa_pool,
            tc.tile_pool(name="pad", bufs=2) as pad_pool,
            tc.tile_pool(name="pz", bufs=3) as pz_pool,
            tc.tile_pool(name="sde", bufs=1) as sde_pool,
            tc.tile_pool(name="qi", bufs=3) as qi_pool,
            tc.tile_pool(name="ed", bufs=3) as ed_pool,
            tc.tile_pool(name="zg", bufs=2) as zg_pool,
            tc.tile_pool(name="wz", bufs=3) as wz_pool,
            tc.tile_pool(name="oh", bufs=3) as oh_pool,
            tc.tile_pool(name="wt", bufs=4) as w_pool,
            tc.tile_pool(name="fl", bufs=4) as f_pool,
            tc.tile_pool(name="ob", bufs=4) as ob_pool,
            tc.tile_pool(name="fg", bufs=1) as fg_pool,
        ):
            wsrc_t = cpool.tile([128, 132], BF16)
            nc.sync.dma_start(out=wsrc_t[:], in_=wsr[:, :])
            wdst_t = cpool.tile([128, 2], BF16)
            nc.sync.dma_start(out=wdst_t[:], in_=wds[:, :])
            iota_t = cpool.tile([128, K * 128], BF16)
            nc.gpsimd.iota(iota_t[:], [[0, K], [1, 128]], channel_multiplier=0,
                           allow_small_or_imprecise_dtypes=True)

            # ---- Phase A1: per-edge s_dst (host pre-expanded h columns) ----
            NC8 = T * K
            sde = sde_pool.tile([128, NC8 * 2], BF16)
            with tc.tile_pool(name="psD", bufs=4, space="PSUM") as psD_pool:
                ng = (NC8 + 63) // 64
                for g in range(ng):
                    lo, hi = g * 64, min((g + 1) * 64, NC8)
                    hTd = pad_pool.tile([128, 64 * 128], BF16)
                    nc.scalar.dma_start(
                        out=hTd[:, 0:(hi - lo) * 128],
                        in_=hdE[:, lo * 128:hi * 128])
                    psd = psD_pool.tile([128, 128], F32)
                    for j in range(hi - lo):
                        nc.tensor.matmul(
                            out=psd[:, j * 2:(j + 1) * 2],
                            lhsT=hTd[:, j * 128:(j + 1) * 128],
                            rhs=wdst_t[:], start=True, stop=True)
                    nc.vector.tensor_copy(
                        out=sde[:, lo * 2:hi * 2], in_=psd[:, 0:(hi - lo) * 2])

            # ---- Phase A2: Z = [z | s_src | pad] for all src nodes ----
            with tc.tile_pool(name="psA", bufs=4, space="PSUM") as psA_pool:
                ci = 0
                for g in range(SRC_TILES // SRC_GROUP):
                    hT = pa_pool.tile([128, SRC_GROUP * 128], BF16, tag="hT")
                    nc.scalar.dma_start(
                        out=hT[:],
                        in_=hsT[:, g * SRC_GROUP * 128:(g + 1) * SRC_GROUP * 128])
                    zbig = pz_pool.tile([128, SRC_GROUP * ZC], BF16)
                    for j in range(SRC_GROUP):
                        ps = psA_pool.tile([128, 132], F32)
                        nc.tensor.matmul(
                            out=ps[:], lhsT=hT[:, j * 128:(j + 1) * 128],
                            rhs=wsrc_t[:], start=True, stop=True)
                        if ci % 2 == 0:
                            nc.vector.tensor_copy(
                                out=zbig[:, j * ZC:j * ZC + 132], in_=ps[:])
                        else:
                            nc.scalar.copy(
                                out=zbig[:, j * ZC:j * ZC + 132], in_=ps[:])
                        ci += 1
                    qq, gl = g // 13, g % 13
                    rows = slice(gl * SRC_GROUP * 128, (gl + 1) * SRC_GROUP * 128)
                    nc.sync.dma_start(
                        out=Zq[qq][rows, :].rearrange("(j p) c -> p j c", p=128),
                        in_=zbig[:].rearrange("p (j c) -> p j c", c=ZC))

            # ---- Phase B: blocks of 8 supertiles ----
            with tc.tile_pool(name="psB", bufs=6, space="PSUM") as psB_pool:
                for b in range(NB):
                    qit = qi_pool.tile([128, NQ * (NSLOT // 16)], I16)
                    nc.sync.dma_start(
                        out=qit[:].rearrange("p (q m) -> p q m", q=NQ),
                        in_=qix[b, :, :, :].rearrange("q p m -> p q m"))
                    edt = ed_pool.tile([128, B * K], BF16)
                    nc.scalar.dma_start(
                        out=edt[:].rearrange("p (s k) -> p s k", k=K),
                        in_=edr[b * B:(b + 1) * B, :, :].rearrange(
                            "s p k -> p s k"))
                    zg = zg_pool.tile([128, (NSLOT // 128) * NQ * ZC], BF16)
                    zg3 = zg[:].rearrange("p (m c) -> p m c", c=ZC)
                    for qq in range(NQ):
                        for g2 in range(NSLOT // GNI):
                            gc, gi = GNI // 128, GNI // 16
                            nc.gpsimd.dma_gather(
                                out_ap=zg3[:, qq * 16 + g2 * gc:
                                           qq * 16 + (g2 + 1) * gc, :],
                                in_ap=Zq[qq][:, :],
                                idxs_ap=qit[:, qq * (NSLOT // 16) + g2 * gi:
                                            qq * (NSLOT // 16) + (g2 + 1) * gi],
                                num_idxs=GNI,
                                num_idxs_reg=GNI,
                                elem_size=ZC,
                            )
                    zg4 = zg[:].rearrange("p (q m c) -> p q m c", q=NQ, c=ZC)
                    for s in range(B):
                        t = b * B + s
                        # chunk j=2q+sub of supertile s -> zg col 16q+s*2+sub
                        sl4 = zg4[:, :, s * 2:s * 2 + 2, :]
                        st = w_pool.tile([128, 2 * K], F32, tag="st")
                        nc.vector.tensor_tensor(
                            out=st[:].rearrange("p (q m c) -> p q m c",
                                                q=NQ, c=2),
                            in0=sl4[:, :, :, IN:IN + 2],
                            in1=sde[:, t * K * 2:(t + 1) * K * 2].rearrange(
                                "p (q m c) -> p q m c", q=NQ, c=2),
                            op=ALU.add)
                        stl = w_pool.tile([128, 2 * K], F32, tag="stl")
                        nc.vector.scalar_tensor_tensor(
                            out=stl[:], in0=st[:], scalar=0.01, in1=st[:],
                            op0=ALU.mult, op1=ALU.max)
                        wt = w_pool.tile([128, 2 * K], BF16, tag="wt")
                        nc.scalar.activation(out=wt[:], in_=stl[:], func=AF.Exp)
                        wt4 = wt[:].rearrange("p (q m c) -> p q m c", q=NQ, c=2)
                        wzb = wz_pool.tile([128, K * 130], BF16)
                        wzb4 = wzb[:].rearrange("p (q m c) -> p q m c",
                                                q=NQ, c=130)
                        for h in range(H):
                            nc.vector.tensor_tensor(
                                out=wzb4[:, :, :, h * OUT:(h + 1) * OUT],
                                in0=sl4[:, :, :, h * OUT:(h + 1) * OUT],
                                in1=wt4[:, :, :, h:h + 1].to_broadcast(
                                    [128, NQ, 2, OUT]),
                                op=ALU.mult)
                        nc.scalar.copy(out=wzb4[:, :, :, IN:IN + 2], in_=wt4[:])
                        oht = oh_pool.tile([128, K * 128], BF16)
                        nc.vector.tensor_tensor(
                            out=oht[:],
                            in0=edt[:, s * K:(s + 1) * K].to_broadcast(
                                [128, K, 128]),
                            in1=iota_t[:].rearrange("p (k q) -> p k q", q=128),
                            op=ALU.is_equal)
                        ps = psB_pool.tile([128, 130], F32)
                        for j in range(K):
                            nc.tensor.matmul(
                                out=ps[:],
                                lhsT=oht[:, j * 128:(j + 1) * 128],
                                rhs=wzb[:, j * 130:(j + 1) * 130],
                                start=(j == 0), stop=(j == K - 1))
                        den = f_pool.tile([128, 2], F32, tag="den")
                        nc.vector.tensor_scalar_max(
                            out=den[:], in0=ps[:, IN:IN + 2], scalar1=1e-30)
                        rec = f_pool.tile([128, 2], F32, tag="rec")
                        nc.vector.reciprocal_approx_fast(out=rec[:], in_=den[:])
                        o0 = f_pool.tile([128, OUT], F32, tag="o0")
                        nc.scalar.mul(o0[:], ps[:, 0:OUT], rec[:, 0:1])
                        ob = ob_pool.tile([128, BIGC], BF16)
                        nc.vector.scalar_tensor_tensor(
                            out=ob[:, 0:OUT], in0=ps[:, OUT:2 * OUT],
                            scalar=rec[:, 1:2], in1=o0[:],
                            op0=ALU.mult, op1=ALU.add)
                        nc.sync.dma_start(
                            out=big[t * 128:(t + 1) * 128, 0:OUT],
                            in_=ob[:, 0:OUT])

                # ---- final: remap slot rows -> dst rows ----
                rmt = qi_pool.tile([128, OUT_ROWS // 16], I16, tag="rmt")
                nc.sync.dma_start(out=rmt[:], in_=rmp[:, :])
                fg = fg_pool.tile([128, (OUT_ROWS // 128) * BIGC], BF16)
                fg3 = fg[:].rearrange("p (m c) -> p m c", c=BIGC)
                pos = 0
                while pos < OUT_ROWS:
                    ni = 1024 if OUT_ROWS - pos >= 1024 else 512
                    nc.gpsimd.dma_gather(
                        out_ap=fg3[:, pos // 128:(pos + ni) // 128, :],
                        in_ap=big[:, :],
                        idxs_ap=rmt[:, pos // 16:(pos + ni) // 16],
                        num_idxs=ni,
                        num_idxs_reg=ni,
                        elem_size=BIGC,
                    )
                    pos += ni
                nc.sync.dma_start(
                    out=out[:, :].rearrange("(m p) c -> p m c", p=128),
                    in_=fg3[:, :, 0:OUT])

    nc.compile()
    return nc


def _prep_inputs(h_src, h_dst, W_src, W_dst, a_w, src_idx, dst_idx):
    hs = np.zeros((SRC_PAD, IN), np.float32)
    hs[:N_SRC] = h_src
    hsrcT = np.ascontiguousarray(hs.T.astype(ml_dtypes.bfloat16))

    wsr = np.zeros((IN, 132), np.float32)
    wsr[:, :H * OUT] = W_src.reshape(H * OUT, IN).T
    a_s, a_d = a_w[:, :OUT], a_w[:, OUT:]
    wsr[:, H * OUT:H * OUT + H] = np.einsum("hod,ho->dh", W_src, a_s)
    wsr = wsr.astype(ml_dtypes.bfloat16)
    wds = np.einsum("hod,ho->dh", W_dst, a_d).astype(ml_dtypes.bfloat16)

    qidx, edrel, dcol, remap, T, NB = _pack_all(
        np.asarray(src_idx), np.asarray(dst_idx))

    in_maps = []
    for c in range(NCORES):
        hd = h_dst[c * NDST_C:(c + 1) * NDST_C].astype(np.float32)
        dc = dcol[c].reshape(T, 128, K).transpose(0, 2, 1).reshape(-1)
        hdEc = np.zeros((T * K * 128, IN), np.float32)
        valid = dc >= 0
        hdEc[valid] = hd[dc[valid]]
        hdEc = np.ascontiguousarray(hdEc.T.astype(ml_dtypes.bfloat16))
        rw = _wrap16(remap[c].astype(np.int16))
        in_maps.append({
            "hsrcT": hsrcT,
            "hdE": hdEc,
            "wsrc": wsr,
            "wdst": wds,
            "qidx": qidx[c],
            "edrel": np.ascontiguousarray(edrel[c]),
            "remap": rw,
        })
    return in_maps, T, NB


def _run(inputs, trace=False):
    inputs = {k: np.asarray(v) for k, v in inputs.items()}
    in_maps, T, NB = _prep_inputs(**inputs)
    nc = _build_program(T, NB)
    res = run_bass_kernel_spmd(
        nc, in_maps, core_ids=list(range(NCORES)), trace=trace)
    parts = [np.asarray(res.results[c]["out"][:NDST_C]).astype(np.float32)
             for c in range(NCORES)]
    return np.concatenate(parts, axis=0), res


def kernel(**inputs):
    out, _ = _run(inputs, trace=False)
    return out
